# revision 21
# baseline (speedup 1.0000x reference)
import os
import numpy as np
from concourse import bass, tile
from concourse import mybir
from concourse.bass_utils import run_bass_kernel_spmd
import bass_rust as _bass_rust

dt = mybir.dt
Alu = mybir.AluOpType
Act = mybir.ActivationFunctionType

N = 4096
F = 512
C = 751
SIDE = 1024
NCORES = 8
RPC = N // NCORES      # 512 rows per core
NT = RPC // 128        # 4 row tiles per core
NEGBIG = np.float32(-1e30)

LAST_EXEC_NS = None


ALL_PARTS = frozenset({"load", "mm", "rank", "xent", "side"})


def _build_program(reps=1, parts=ALL_PARTS):
    nc = bass.Bass()
    xtp_d = [nc.dram_tensor(f"xtp{k}", [128, N + RPC], dt.float32r,
                            kind="ExternalInput") for k in range(4)]
    colsq_d = nc.dram_tensor("colsq", [128, N], dt.float32,
                             kind="ExternalInput")
    sqi_d = nc.dram_tensor("sqi", [128, NT], dt.float32,
                           kind="ExternalInput")
    ms_d = nc.dram_tensor("mstrip", [128, 1800], dt.float32, kind="ExternalInput")
    cls_d = nc.dram_tensor("cls", [RPC, C], dt.float32, kind="ExternalInput")
    l2_d = nc.dram_tensor("l2", [RPC, SIDE], dt.float32, kind="ExternalInput")
    l3_d = nc.dram_tensor("l3", [RPC, SIDE], dt.float32, kind="ExternalInput")
    l4_d = nc.dram_tensor("l4", [RPC, SIDE], dt.float32, kind="ExternalInput")
    out_d = nc.dram_tensor("out", [128, 18], dt.float32, kind="ExternalOutput")

    with tile.TileContext(nc) as tc:
        with tc.tile_pool(name="sb", bufs=1) as sb, \
             tc.tile_pool(name="ps", bufs=8, space="PSUM") as ps:
            xtp_t = [sb.tile([128, N + RPC], dt.float32r, name=f"xtp_t{k}")
                     for k in range(4)]
            colsq_t = sb.tile([128, N], dt.float32)
            sqi_t = sb.tile([128, NT], dt.float32)
            ms_t = sb.tile([128, 1800], dt.float32)
            cls_t = [sb.tile([128, C], dt.float32, name=f"cls_t{r}")
                     for r in range(NT)]
            l2_t = [sb.tile([128, SIDE], dt.float32, name=f"l2_t{r}")
                    for r in range(NT)]
            l3_t = [sb.tile([128, SIDE], dt.float32, name=f"l3_t{r}")
                    for r in range(NT)]
            l4_t = [sb.tile([128, SIDE], dt.float32, name=f"l4_t{r}")
                    for r in range(NT)]
            out_t = sb.tile([128, 18], dt.float32)
            a2 = sb.tile([128, NT], dt.float32)
            a3 = sb.tile([128, NT], dt.float32)

            tmpA = sb.tile([128, 512], dt.float32)
            tmpB = sb.tile([128, 512], dt.float32)
            tmpC = sb.tile([128, 512], dt.float32)
            cand = sb.tile([128, 64], dt.float32)
            pos8 = sb.tile([128, 8], dt.float32)
            neg8 = sb.tile([128, 8], dt.float32)
            pos8r = sb.tile([128, 8], dt.float32)
            cmp = sb.tile([128, 8], dt.float32)
            m_t = sb.tile([128, 1], dt.float32)
            clampP0 = sb.tile([128, 8], dt.float32)
            clampP = sb.tile([128, 8], dt.float32)
            pP = sb.tile([128, 8], dt.float32)
            clampN0 = sb.tile([128, 8], dt.float32)
            clampN = sb.tile([128, 8], dt.float32)
            nN = sb.tile([128, 8], dt.float32)
            n0e = sb.tile([128, 1], dt.float32)
            rec = sb.tile([128, 1], dt.float32)
            dlt = sb.tile([128, 8], dt.float32)
            rat = sb.tile([128, 8], dt.float32)
            E = sb.tile([128, 8], dt.float32)
            w0 = sb.tile([128, 8], dt.float32)
            ind = sb.tile([128, 8], dt.float32)
            diff = sb.tile([128, 8], dt.float32)
            t1 = sb.tile([128, 8], dt.float32)
            t2 = sb.tile([128, 8], dt.float32)
            t3 = sb.tile([128, 8], dt.float32)
            t5 = sb.tile([128, 8], dt.float32)
            l8 = sb.tile([128, 8], dt.float32)
            negmax = sb.tile([128, 1], dt.float32)
            scr = sb.tile([128, C], dt.float32)
            se = sb.tile([128, 1], dt.float32)
            lse = sb.tile([128, 1], dt.float32)
            sd2 = sb.tile([128, SIDE], dt.float32)
            sq2 = sb.tile([128, SIDE], dt.float32)
            sd3 = sb.tile([128, SIDE], dt.float32)
            sq3 = sb.tile([128, SIDE], dt.float32)

            kt = ms_t[:, 1792:1800]

            if parts != ALL_PARTS:
                for k in range(4):
                    nc.gpsimd.dma_start(xtp_t[k][:], xtp_d[k][:])
                nc.gpsimd.dma_start(colsq_t[:], colsq_d[:])
                nc.gpsimd.dma_start(sqi_t[:], sqi_d[:])
                nc.gpsimd.dma_start(ms_t[:], ms_d[:])
                for r in range(NT):
                    nc.gpsimd.dma_start(cls_t[r][:],
                                        cls_d[128 * r:128 * r + 128, :])
                    nc.gpsimd.dma_start(l2_t[r][:],
                                        l2_d[128 * r:128 * r + 128, :])
                    nc.gpsimd.dma_start(l3_t[r][:],
                                        l3_d[128 * r:128 * r + 128, :])
                    nc.gpsimd.dma_start(l4_t[r][:],
                                        l4_d[128 * r:128 * r + 128, :])
                for t in (out_t, cand, pos8):
                    nc.vector.memset(t[:], 0.25)

            for _ in range(reps):
                if "load" in parts:
                    for k in range(4):
                        nc.gpsimd.dma_start(xtp_t[k][:], xtp_d[k][:])
                    nc.gpsimd.dma_start(colsq_t[:], colsq_d[:])
                    nc.gpsimd.dma_start(sqi_t[:], sqi_d[:])
                    nc.gpsimd.dma_start(ms_t[:], ms_d[:])
                    for r in range(NT):
                        nc.gpsimd.dma_start(cls_t[r][:],
                                            cls_d[128 * r:128 * r + 128, :])
                        nc.gpsimd.dma_start(l2_t[r][:],
                                            l2_d[128 * r:128 * r + 128, :])
                        nc.gpsimd.dma_start(l3_t[r][:],
                                            l3_d[128 * r:128 * r + 128, :])
                        nc.gpsimd.dma_start(l4_t[r][:],
                                            l4_d[128 * r:128 * r + 128, :])

                for r in range(NT):
                    lT = slice(N + 128 * r, N + 128 * r + 128)
                    if "mm" in parts:
                        for cb in range(8):
                            cS = slice(512 * cb, 512 * cb + 512)
                            p = ps.tile([128, 512], dt.float32, name="p")
                            for k in range(4):
                                nc.tensor.matmul(p[:], xtp_t[k][:, lT],
                                                 xtp_t[k][:, cS],
                                                 start=(k == 0),
                                                 stop=(k == 3))
                            nc.vector.tensor_tensor(tmpC[:], p[:],
                                                    colsq_t[:, cS], Alu.add)
                            if cb == 0:
                                nc.vector.tensor_tensor(
                                    tmpA[:], tmpC[:],
                                    ms_t[:, 384 - 128 * r:896 - 128 * r],
                                    Alu.add)
                                nc.vector.max(pos8[:], tmpA[:])
                                nc.vector.tensor_tensor(
                                    tmpB[:], tmpC[:],
                                    ms_t[:, 1280 - 128 * r:1792 - 128 * r],
                                    Alu.add)
                                nc.vector.max(cand[:, 0:8], tmpB[:])
                            else:
                                nc.vector.max(cand[:, 8 * cb:8 * cb + 8],
                                              tmpC[:])

                    if "rank" in parts:
                        nc.vector.max(neg8[:], cand[:])
                        nc.vector.tensor_scalar_add(pos8r[:], pos8[:, 7::-1],
                                                    0.0)
                        nc.vector.tensor_tensor(cmp[:], neg8[:], pos8r[:],
                                                Alu.is_gt)
                        nc.vector.tensor_reduce(m_t[:], cmp[:],
                                                mybir.AxisListType.X, Alu.add)
                        nc.vector.tensor_scalar(clampP0[:], pos8r[:],
                                                sqi_t[:, r:r + 1], -1.0,
                                                Alu.subtract, Alu.mult)
                        nc.vector.tensor_scalar(clampP[:], clampP0[:], 1e-12,
                                                0.0, Alu.max, Alu.add)
                        nc.scalar.activation(pP[:], clampP[:], Act.Sqrt)
                        nc.vector.tensor_scalar(clampN0[:], neg8[:],
                                                sqi_t[:, r:r + 1], -1.0,
                                                Alu.subtract, Alu.mult)
                        nc.vector.tensor_scalar(clampN[:], clampN0[:], 1e-12,
                                                0.0, Alu.max, Alu.add)
                        nc.scalar.activation(nN[:], clampN[:], Act.Sqrt)
                        nc.vector.tensor_scalar_add(n0e[:], nN[:, 0:1], 1e-12)
                        nc.vector.reciprocal(rec[:], n0e[:])
                        nc.vector.tensor_scalar(dlt[:], nN[:], nN[:, 0:1],
                                                -1.0, Alu.subtract, Alu.mult)
                        nc.vector.tensor_scalar(rat[:], dlt[:], rec[:], 0.0,
                                                Alu.mult, Alu.add)
                        nc.scalar.activation(E[:], rat[:], Act.Exp)
                        nc.vector.tensor_scalar(w0[:], kt, m_t[:], -1.0,
                                                Alu.subtract, Alu.mult)
                        nc.vector.tensor_scalar(ind[:], w0[:], 0.0, 1.0,
                                                Alu.max, Alu.min)
                        nc.vector.tensor_tensor(diff[:], pP[:], nN[:],
                                                Alu.subtract)
                        nc.vector.tensor_tensor(t1[:], E[:], diff[:],
                                                Alu.mult)
                        nc.vector.tensor_tensor(t2[:], t1[:], w0[:], Alu.mult)
                        nc.vector.tensor_tensor(t3[:], t2[:], ind[:],
                                                Alu.mult)
                        nc.vector.tensor_scalar(t5[:], ind[:], 0.5, 0.0,
                                                Alu.mult, Alu.add)
                        nc.vector.tensor_tensor(l8[:], t3[:], t5[:], Alu.add)
                        nc.vector.tensor_reduce(out_t[:, 4 * r:4 * r + 1],
                                                l8[:], mybir.AxisListType.X,
                                                Alu.add)
                        nc.vector.tensor_scalar_add(
                            out_t[:, 4 * r + 1:4 * r + 2], m_t[:], 0.0)

                    if "xent" in parts:
                        nc.vector.tensor_reduce(negmax[:], cls_t[r][:],
                                                mybir.AxisListType.X, Alu.max,
                                                negate=True)
                        nc.scalar.activation(scr[:], cls_t[r][:], Act.Exp,
                                             bias=negmax[:], scale=1.0,
                                             accum_out=se[:])
                        nc.scalar.activation(lse[:], se[:], Act.Ln)
                        nc.vector.tensor_tensor(out_t[:, 4 * r + 2:4 * r + 3],
                                                lse[:], negmax[:],
                                                Alu.subtract)

                    if "side" in parts:
                        nc.vector.tensor_tensor(sd2[:], l4_t[r][:],
                                                l2_t[r][:], Alu.subtract)
                        nc.scalar.activation(sq2[:], sd2[:], Act.Square,
                                             accum_out=a2[:, r:r + 1])
                        nc.vector.tensor_tensor(sd3[:], l4_t[r][:],
                                                l3_t[r][:], Alu.subtract)
                        nc.scalar.activation(sq3[:], sd3[:], Act.Square,
                                             accum_out=a3[:, r:r + 1])

                if "side" in parts:
                    nc.vector.tensor_reduce(out_t[:, 16:17], a2[:],
                                            mybir.AxisListType.X, Alu.add)
                    nc.vector.tensor_reduce(out_t[:, 17:18], a3[:],
                                            mybir.AxisListType.X, Alu.add)
                nc.sync.dma_start(out_d[:], out_t[:])

    _bass_rust.move_matmul_waits_to_ldweights(nc.m)
    _bass_rust.generate_event_semaphores(nc)
    return nc


def _build_mstrip():
    K0 = np.full((128, 128), NEGBIG, np.float32)
    D0 = np.zeros((128, 128), np.float32)
    for b in range(16):
        K0[8 * b:8 * b + 8, 8 * b:8 * b + 8] = 0.0
        D0[8 * b:8 * b + 8, 8 * b:8 * b + 8] = NEGBIG
    kstrip = np.full((128, 896), NEGBIG, np.float32)
    kstrip[:, 384:512] = K0
    drop = np.zeros((128, 896), np.float32)
    drop[:, 384:512] = D0
    kv = np.tile(np.arange(8, dtype=np.float32), (128, 1))
    return np.ascontiguousarray(np.concatenate([kstrip, drop, kv], axis=1))


def _make_in_maps(cls_fea, l2, l3, l4, x):
    sq = (x.astype(np.float64) ** 2).sum(1).astype(np.float32)
    xT = np.ascontiguousarray(x.T)
    mstrip = _build_mstrip()

    in_maps = []
    for c in range(NCORES):
        R0 = RPC * c
        perm = np.concatenate([np.arange(R0, R0 + RPC),
                               np.arange(0, R0),
                               np.arange(R0 + RPC, N)])
        xt_perm = xT[:, perm]
        im = {}
        for k in range(4):
            im[f"xtp{k}"] = np.ascontiguousarray(np.concatenate(
                [xt_perm[128 * k:128 * k + 128, :],
                 2.0 * xt_perm[128 * k:128 * k + 128, 0:RPC]], axis=1))
        im["colsq"] = np.ascontiguousarray(np.broadcast_to(
            (-sq[perm]).reshape(1, N), (128, N))).astype(np.float32)
        sqi = np.empty((128, NT), np.float32)
        for r in range(NT):
            sqi[:, r] = sq[R0 + 128 * r:R0 + 128 * r + 128]
        im["sqi"] = sqi
        im["mstrip"] = mstrip
        im["cls"] = np.ascontiguousarray(cls_fea[R0:R0 + RPC])
        im["l2"] = np.ascontiguousarray(l2[R0:R0 + RPC])
        im["l3"] = np.ascontiguousarray(l3[R0:R0 + RPC])
        im["l4"] = np.ascontiguousarray(l4[R0:R0 + RPC])
        in_maps.append(im)
    return in_maps


def _postprocess(results, cls_fea, x, targets):
    losses = np.empty(N, np.float64)
    ms = np.empty(N, np.float64)
    lse = np.empty(N, np.float64)
    s2 = 0.0
    s3 = 0.0
    for c in range(NCORES):
        o = np.asarray(results[c]["out"], np.float64)
        for r in range(NT):
            rows = slice(RPC * c + 128 * r, RPC * c + 128 * r + 128)
            losses[rows] = o[:, 4 * r]
            ms[rows] = o[:, 4 * r + 1]
            lse[rows] = o[:, 4 * r + 2]
        s2 += float(o[:, 16].sum())
        s3 += float(o[:, 17].sum())

    rank_loss = losses.sum() / N
    prec = float((ms < 0.5).mean())
    gathered = cls_fea[np.arange(N), targets].astype(np.float64)
    xent = float((lse - gathered).mean())
    side = np.sqrt(s2) + np.sqrt(s3)
    acc = float((np.argmax(x, axis=1).astype(np.int64) == targets).mean())
    total = rank_loss + xent + 0.1 * side
    prec2 = max(prec, acc)
    return np.array([total, prec2], np.float32)


def kernel(**inputs):
    global LAST_EXEC_NS
    cls_fea = np.ascontiguousarray(np.asarray(inputs["cls_fea"], np.float32))
    l2 = np.asarray(inputs["l2_side"], np.float32)
    l3 = np.asarray(inputs["l3_side"], np.float32)
    l4 = np.asarray(inputs["l4_side"], np.float32)
    x = np.asarray(inputs["input_fea"], np.float32)
    targets = np.asarray(inputs["targets"]).astype(np.int64)

    in_maps = _make_in_maps(cls_fea, l2, l3, l4, x)
    nc = _build_program()
    trace = os.environ.get("KERNEL_TRACE", "0") == "1"
    res = run_bass_kernel_spmd(nc, in_maps, list(range(NCORES)), trace=trace)
    LAST_EXEC_NS = res.exec_time_ns
    return _postprocess(res.results, cls_fea, x, targets)


# revision 24
# speedup vs baseline: 1.4383x; 1.4383x over previous
import os
import numpy as np
from concourse import bass, tile
from concourse import mybir
from concourse.bass_utils import run_bass_kernel_spmd
import bass_rust as _bass_rust

dt = mybir.dt
Alu = mybir.AluOpType
Act = mybir.ActivationFunctionType

N = 4096
F = 512
C = 751
SIDE = 1024
NCORES = 8
RPC = N // NCORES      # 512 rows per core
NT = RPC // 128        # 4 row tiles per core
NMOV = 3               # moving-block ring depth
NEGBIG = np.float32(-1e30)

LAST_EXEC_NS = None


ALL_PARTS = frozenset({"load", "mm", "rank", "xent", "side"})


def _build_program(reps=1, parts=ALL_PARTS):
    nc = bass.Bass()
    xtp_d = [nc.dram_tensor(f"xtp{k}", [128, N], dt.float32r,
                            kind="ExternalInput") for k in range(4)]
    colsq_d = nc.dram_tensor("colsq", [128, N], dt.float32,
                             kind="ExternalInput")
    sqi_d = nc.dram_tensor("sqi", [128, NT], dt.float32,
                           kind="ExternalInput")
    ms_d = nc.dram_tensor("mstrip", [128, 1800], dt.float32,
                          kind="ExternalInput")
    cls_d = nc.dram_tensor("cls", [RPC, C], dt.float32, kind="ExternalInput")
    l2_d = nc.dram_tensor("l2", [RPC, SIDE], dt.float32, kind="ExternalInput")
    l3_d = nc.dram_tensor("l3", [RPC, SIDE], dt.float32, kind="ExternalInput")
    l4_d = nc.dram_tensor("l4", [RPC, SIDE], dt.float32, kind="ExternalInput")
    out_d = nc.dram_tensor("out", [128, 18], dt.float32, kind="ExternalOutput")

    with tile.TileContext(nc) as tc:
        with tc.tile_pool(name="sb", bufs=1) as sb, \
             tc.tile_pool(name="ps", bufs=8, space="PSUM") as ps:
            stat_t = [[sb.tile([128, 512], dt.float32r, name=f"stat{b}_{k}")
                       for k in range(4)] for b in range(2)]
            mov_t = [[sb.tile([128, 512], dt.float32r, name=f"mov{s}_{k}")
                      for k in range(4)] for s in range(NMOV)]
            colsq_t = [sb.tile([128, N], dt.float32, name=f"colsq{b}")
                       for b in range(2)]
            sqi_t = [sb.tile([128, NT], dt.float32, name=f"sqi{b}")
                     for b in range(2)]
            ms_t = [sb.tile([128, 1800], dt.float32, name=f"ms{b}")
                    for b in range(2)]
            cls_t = [sb.tile([128, C], dt.float32, name=f"cls_t{r}")
                     for r in range(NT)]
            l2_t = [sb.tile([128, SIDE], dt.float32, name=f"l2_t{r}")
                    for r in range(NT)]
            l3_t = [sb.tile([128, SIDE], dt.float32, name=f"l3_t{r}")
                    for r in range(NT)]
            l4_t = [sb.tile([128, SIDE], dt.float32, name=f"l4_t{r}")
                    for r in range(NT)]
            out_t = [sb.tile([128, 18], dt.float32, name=f"out{b}")
                     for b in range(2)]
            a2 = [sb.tile([128, NT], dt.float32, name=f"a2_{b}")
                  for b in range(2)]
            a3 = [sb.tile([128, NT], dt.float32, name=f"a3_{b}")
                  for b in range(2)]

            tmpA = sb.tile([128, 512], dt.float32)
            tmpB = sb.tile([128, 512], dt.float32)
            tmpC = sb.tile([128, 512], dt.float32)
            cand_r = [sb.tile([128, 64], dt.float32, name=f"cand{r}")
                      for r in range(NT)]
            pos8_r = [sb.tile([128, 8], dt.float32, name=f"pos8_{r}")
                      for r in range(NT)]
            neg8 = sb.tile([128, 8], dt.float32)
            pos8r = sb.tile([128, 8], dt.float32)
            cmp = sb.tile([128, 8], dt.float32)
            m_t = sb.tile([128, 1], dt.float32)
            clampP0 = sb.tile([128, 8], dt.float32)
            clampP = sb.tile([128, 8], dt.float32)
            pP = sb.tile([128, 8], dt.float32)
            clampN0 = sb.tile([128, 8], dt.float32)
            clampN = sb.tile([128, 8], dt.float32)
            nN = sb.tile([128, 8], dt.float32)
            n0e = sb.tile([128, 1], dt.float32)
            rec = sb.tile([128, 1], dt.float32)
            dlt = sb.tile([128, 8], dt.float32)
            rat = sb.tile([128, 8], dt.float32)
            E = sb.tile([128, 8], dt.float32)
            w0 = sb.tile([128, 8], dt.float32)
            ind = sb.tile([128, 8], dt.float32)
            diff = sb.tile([128, 8], dt.float32)
            t1 = sb.tile([128, 8], dt.float32)
            t2 = sb.tile([128, 8], dt.float32)
            t3 = sb.tile([128, 8], dt.float32)
            t5 = sb.tile([128, 8], dt.float32)
            l8 = sb.tile([128, 8], dt.float32)
            negmax = sb.tile([128, 1], dt.float32)
            scr = sb.tile([128, C], dt.float32)
            se = sb.tile([128, 1], dt.float32)
            lse = sb.tile([128, 1], dt.float32)
            sd2 = sb.tile([128, SIDE], dt.float32)
            sq2 = sb.tile([128, SIDE], dt.float32)
            sd3 = sb.tile([128, SIDE], dt.float32)
            sq3 = sb.tile([128, SIDE], dt.float32)

            if parts != ALL_PARTS:
                for b in range(2):
                    for k in range(4):
                        nc.gpsimd.dma_start(stat_t[b][k][:],
                                            xtp_d[k][:, 0:512])
                    nc.scalar.dma_start(colsq_t[b][:], colsq_d[:])
                    nc.scalar.dma_start(sqi_t[b][:], sqi_d[:])
                    nc.scalar.dma_start(ms_t[b][:], ms_d[:])
                for s in range(NMOV):
                    for k in range(4):
                        nc.gpsimd.dma_start(mov_t[s][k][:],
                                            xtp_d[k][:, 0:512])
                for r in range(NT):
                    nc.scalar.dma_start(cls_t[r][:],
                                        cls_d[128 * r:128 * r + 128, :])
                    nc.sync.dma_start(l2_t[r][:],
                                      l2_d[128 * r:128 * r + 128, :])
                    nc.sync.dma_start(l3_t[r][:],
                                      l3_d[128 * r:128 * r + 128, :])
                    nc.sync.dma_start(l4_t[r][:],
                                      l4_d[128 * r:128 * r + 128, :])
                for b in range(2):
                    nc.vector.memset(out_t[b][:], 0.25)
                for r in range(NT):
                    nc.vector.memset(cand_r[r][:], 0.25)
                    nc.vector.memset(pos8_r[r][:], 0.25)

            for rep in range(reps):
                b = rep % 2
                stat = stat_t[b]
                colsq = colsq_t[b]
                sqi = sqi_t[b]
                ms = ms_t[b]
                out = out_t[b]
                kt = ms[:, 1792:1800]

                if "load" in parts:
                    for k in range(4):
                        nc.gpsimd.dma_start(stat[k][:], xtp_d[k][:, 0:512])
                    nc.scalar.dma_start(colsq[:], colsq_d[:])
                    nc.scalar.dma_start(sqi[:], sqi_d[:])
                    nc.scalar.dma_start(ms[:], ms_d[:])
                    for r in range(NT):
                        nc.scalar.dma_start(cls_t[r][:],
                                            cls_d[128 * r:128 * r + 128, :])
                        nc.sync.dma_start(l2_t[r][:],
                                          l2_d[128 * r:128 * r + 128, :])
                        nc.sync.dma_start(l3_t[r][:],
                                          l3_d[128 * r:128 * r + 128, :])
                        nc.sync.dma_start(l4_t[r][:],
                                          l4_d[128 * r:128 * r + 128, :])

                for cb in range(8):
                    cS = slice(512 * cb, 512 * cb + 512)
                    if cb == 0:
                        mov = stat
                    else:
                        mov = mov_t[(cb - 1) % NMOV]
                        if "load" in parts:
                            for k in range(4):
                                nc.gpsimd.dma_start(mov[k][:], xtp_d[k][:, cS])
                    if "mm" in parts:
                        for r in range(NT):
                            lS = slice(128 * r, 128 * r + 128)
                            p = ps.tile([128, 512], dt.float32, name="p")
                            for k in range(4):
                                nc.tensor.matmul(p[:], stat[k][:, lS],
                                                 mov[k][:],
                                                 start=(k == 0),
                                                 stop=(k == 3))
                            nc.vector.tensor_tensor(tmpC[:], p[:],
                                                    colsq[:, cS], Alu.add)
                            if cb == 0:
                                nc.vector.tensor_tensor(
                                    tmpA[:], tmpC[:],
                                    ms[:, 384 - 128 * r:896 - 128 * r],
                                    Alu.add)
                                nc.vector.max(pos8_r[r][:], tmpA[:])
                                nc.vector.tensor_tensor(
                                    tmpB[:], tmpC[:],
                                    ms[:, 1280 - 128 * r:1792 - 128 * r],
                                    Alu.add)
                                nc.vector.max(cand_r[r][:, 0:8], tmpB[:])
                            else:
                                nc.vector.max(cand_r[r][:, 8 * cb:8 * cb + 8],
                                              tmpC[:])

                for r in range(NT):
                    if "rank" in parts:
                        nc.vector.max(neg8[:], cand_r[r][:])
                        nc.vector.tensor_scalar_add(pos8r[:],
                                                    pos8_r[r][:, 7::-1], 0.0)
                        nc.vector.tensor_tensor(cmp[:], neg8[:], pos8r[:],
                                                Alu.is_gt)
                        nc.vector.tensor_reduce(m_t[:], cmp[:],
                                                mybir.AxisListType.X, Alu.add)
                        nc.vector.tensor_scalar(clampP0[:], pos8r[:],
                                                sqi[:, r:r + 1], -2.0,
                                                Alu.subtract, Alu.mult)
                        nc.vector.tensor_scalar(clampP[:], clampP0[:], 1e-12,
                                                0.0, Alu.max, Alu.add)
                        nc.scalar.activation(pP[:], clampP[:], Act.Sqrt)
                        nc.vector.tensor_scalar(clampN0[:], neg8[:],
                                                sqi[:, r:r + 1], -2.0,
                                                Alu.subtract, Alu.mult)
                        nc.vector.tensor_scalar(clampN[:], clampN0[:], 1e-12,
                                                0.0, Alu.max, Alu.add)
                        nc.scalar.activation(nN[:], clampN[:], Act.Sqrt)
                        nc.vector.tensor_scalar_add(n0e[:], nN[:, 0:1], 1e-12)
                        nc.vector.reciprocal(rec[:], n0e[:])
                        nc.vector.tensor_scalar(dlt[:], nN[:], nN[:, 0:1],
                                                -1.0, Alu.subtract, Alu.mult)
                        nc.vector.tensor_scalar(rat[:], dlt[:], rec[:], 0.0,
                                                Alu.mult, Alu.add)
                        nc.scalar.activation(E[:], rat[:], Act.Exp)
                        nc.vector.tensor_scalar(w0[:], kt, m_t[:], -1.0,
                                                Alu.subtract, Alu.mult)
                        nc.vector.tensor_scalar(ind[:], w0[:], 0.0, 1.0,
                                                Alu.max, Alu.min)
                        nc.vector.tensor_tensor(diff[:], pP[:], nN[:],
                                                Alu.subtract)
                        nc.vector.tensor_tensor(t1[:], E[:], diff[:],
                                                Alu.mult)
                        nc.vector.tensor_tensor(t2[:], t1[:], w0[:], Alu.mult)
                        nc.vector.tensor_tensor(t3[:], t2[:], ind[:],
                                                Alu.mult)
                        nc.vector.tensor_scalar(t5[:], ind[:], 0.5, 0.0,
                                                Alu.mult, Alu.add)
                        nc.vector.tensor_tensor(l8[:], t3[:], t5[:], Alu.add)
                        nc.vector.tensor_reduce(out[:, 4 * r:4 * r + 1],
                                                l8[:], mybir.AxisListType.X,
                                                Alu.add)
                        nc.vector.tensor_scalar_add(
                            out[:, 4 * r + 1:4 * r + 2], m_t[:], 0.0)

                    if "xent" in parts:
                        nc.vector.tensor_reduce(negmax[:], cls_t[r][:],
                                                mybir.AxisListType.X, Alu.max,
                                                negate=True)
                        nc.scalar.activation(scr[:], cls_t[r][:], Act.Exp,
                                             bias=negmax[:], scale=1.0,
                                             accum_out=se[:])
                        nc.scalar.activation(lse[:], se[:], Act.Ln)
                        nc.vector.tensor_tensor(out[:, 4 * r + 2:4 * r + 3],
                                                lse[:], negmax[:],
                                                Alu.subtract)

                    if "side" in parts:
                        nc.vector.tensor_tensor(sd2[:], l4_t[r][:],
                                                l2_t[r][:], Alu.subtract)
                        nc.scalar.activation(sq2[:], sd2[:], Act.Square,
                                             accum_out=a2[b][:, r:r + 1])
                        nc.vector.tensor_tensor(sd3[:], l4_t[r][:],
                                                l3_t[r][:], Alu.subtract)
                        nc.scalar.activation(sq3[:], sd3[:], Act.Square,
                                             accum_out=a3[b][:, r:r + 1])

                if "side" in parts:
                    nc.vector.tensor_reduce(out[:, 16:17], a2[b][:],
                                            mybir.AxisListType.X, Alu.add)
                    nc.vector.tensor_reduce(out[:, 17:18], a3[b][:],
                                            mybir.AxisListType.X, Alu.add)
                nc.sync.dma_start(out_d[:], out[:])

    _bass_rust.move_matmul_waits_to_ldweights(nc.m)
    _bass_rust.generate_event_semaphores(nc)
    return nc


def _build_mstrip():
    K0 = np.full((128, 128), NEGBIG, np.float32)
    D0 = np.zeros((128, 128), np.float32)
    for b in range(16):
        K0[8 * b:8 * b + 8, 8 * b:8 * b + 8] = 0.0
        D0[8 * b:8 * b + 8, 8 * b:8 * b + 8] = NEGBIG
    kstrip = np.full((128, 896), NEGBIG, np.float32)
    kstrip[:, 384:512] = K0
    drop = np.zeros((128, 896), np.float32)
    drop[:, 384:512] = D0
    kv = np.tile(np.arange(8, dtype=np.float32), (128, 1))
    return np.ascontiguousarray(np.concatenate([kstrip, drop, kv], axis=1))


def _make_in_maps(cls_fea, l2, l3, l4, x):
    sq = (x.astype(np.float64) ** 2).sum(1).astype(np.float32)
    xT = np.ascontiguousarray(x.T)
    mstrip = _build_mstrip()

    in_maps = []
    for c in range(NCORES):
        R0 = RPC * c
        perm = np.concatenate([np.arange(R0, R0 + RPC),
                               np.arange(0, R0),
                               np.arange(R0 + RPC, N)])
        xt_perm = xT[:, perm]
        im = {}
        for k in range(4):
            im[f"xtp{k}"] = np.ascontiguousarray(
                xt_perm[128 * k:128 * k + 128, :])
        im["colsq"] = np.ascontiguousarray(np.broadcast_to(
            (-0.5 * sq[perm]).reshape(1, N), (128, N))).astype(np.float32)
        sqi = np.empty((128, NT), np.float32)
        for r in range(NT):
            sqi[:, r] = 0.5 * sq[R0 + 128 * r:R0 + 128 * r + 128]
        im["sqi"] = sqi
        im["mstrip"] = mstrip
        im["cls"] = np.ascontiguousarray(cls_fea[R0:R0 + RPC])
        im["l2"] = np.ascontiguousarray(l2[R0:R0 + RPC])
        im["l3"] = np.ascontiguousarray(l3[R0:R0 + RPC])
        im["l4"] = np.ascontiguousarray(l4[R0:R0 + RPC])
        in_maps.append(im)
    return in_maps


def _postprocess(results, cls_fea, x, targets):
    losses = np.empty(N, np.float64)
    ms = np.empty(N, np.float64)
    lse = np.empty(N, np.float64)
    s2 = 0.0
    s3 = 0.0
    for c in range(NCORES):
        o = np.asarray(results[c]["out"], np.float64)
        for r in range(NT):
            rows = slice(RPC * c + 128 * r, RPC * c + 128 * r + 128)
            losses[rows] = o[:, 4 * r]
            ms[rows] = o[:, 4 * r + 1]
            lse[rows] = o[:, 4 * r + 2]
        s2 += float(o[:, 16].sum())
        s3 += float(o[:, 17].sum())

    rank_loss = losses.sum() / N
    prec = float((ms < 0.5).mean())
    gathered = cls_fea[np.arange(N), targets].astype(np.float64)
    xent = float((lse - gathered).mean())
    side = np.sqrt(s2) + np.sqrt(s3)
    acc = float((np.argmax(x, axis=1).astype(np.int64) == targets).mean())
    total = rank_loss + xent + 0.1 * side
    prec2 = max(prec, acc)
    return np.array([total, prec2], np.float32)


def kernel(**inputs):
    global LAST_EXEC_NS
    cls_fea = np.ascontiguousarray(np.asarray(inputs["cls_fea"], np.float32))
    l2 = np.asarray(inputs["l2_side"], np.float32)
    l3 = np.asarray(inputs["l3_side"], np.float32)
    l4 = np.asarray(inputs["l4_side"], np.float32)
    x = np.asarray(inputs["input_fea"], np.float32)
    targets = np.asarray(inputs["targets"]).astype(np.int64)

    in_maps = _make_in_maps(cls_fea, l2, l3, l4, x)
    nc = _build_program()
    trace = os.environ.get("KERNEL_TRACE", "0") == "1"
    res = run_bass_kernel_spmd(nc, in_maps, list(range(NCORES)), trace=trace)
    LAST_EXEC_NS = res.exec_time_ns
    return _postprocess(res.results, cls_fea, x, targets)
